# revision 23
# baseline (speedup 1.0000x reference)
"""Trainium2 Bass kernel for 16-head MHA (B=2, T=2048, E=1024), SPMD on 8 cores.

Sharding: data-parallel over batch (2) x tensor-parallel over heads (4 groups
of 4 heads). Each core computes, for its (batch b, head-group g):
  qk^T projection (feature-major), v projection (token-major, packed with
  ones/zeros columns for the softmax denominator), shifted-softmax attention
  via an augmented-row matmul trick, and a partial output projection over its
  256 embedding columns. The host sums the 4 partial projections per batch.

All matmul operands are float32r (fp32 bits, PE runs 1 cycle/row vs 4 for
plain fp32 when the moving dim is >= 256).

Softmax shift: the exact per-query max over all keys is computed on-device
(q-stationary matmul in [i, j] orientation + wide free-dim DVE reduces with
negate=True) and folded into the main QK^T matmul as a rank-1 augmented row,
so scores arrive in PSUM already shifted: S'[j,i] = 8*q_i.k_j - M_i. exp()
runs on ACT straight out of PSUM. The softmax denominator comes from ones
columns pre-planted in the packed V tile.

Engine overlap: pass-0 (PE-light, DVE-heavy) of head h+1 is emitted
interleaved into the main pass (PE-heavy) of head h; pass-0 of head 0
interleaves with the V projection; the output projection interleaves with
head 3's main pass (PSUM pools are re-scoped so the banks fit).
"""

import sys

sys.path.insert(0, "/opt/trn_rl_repo")

import numpy as np

import concourse.bass as bass
import concourse.mybir as mybir
import concourse.tile as tile_mod

F32 = mybir.dt.float32
F32R = mybir.dt.float32r
F8 = mybir.dt.float8e5
BF16 = mybir.dt.bfloat16
DR = mybir.MatmulPerfMode.DoubleRow
AX = mybir.AxisListType.X

B, T, E = 2, 2048, 1024
H_TOTAL, D = 16, 64
N_CORES = 8
GROUPS = 4          # head-group (tensor) parallelism
HPG = H_TOTAL // GROUPS  # 4 heads per group
DV = HPG * D        # 256: v width / out-proj contraction per core
FQK = 2 * DV        # 512: q+k feature rows per core
SCALE = float(np.sqrt(D))  # reference MULTIPLIES scores by sqrt(d)

NE = E // 128       # 8 e-chunks
NT_TILE = T // 128  # 16 token tiles
NT_CHUNK = T // 512  # 4 token chunks

# packed V layout per j-tile: [v_h0(64) 1 | v_h2(64) 1 | 1 0*63 v_h1(64) | 1 0*63 v_h3(64)]
# even heads: stationary 65 cols -> PV out rows 0..63 = O, row 64 = denominator
# odd heads: stationary 128 cols -> PV out row 0 = denominator, rows 64..127 = O
VSTRIDE = 386
V_OFF = {0: 0, 2: 65, 1: 194, 3: 322}       # v column base per head
V_STAT = {0: (0, 65), 2: (65, 130), 1: (130, 258), 3: (258, 386)}
ONES_COLS = (64, 129, 130, 258)
ZERO_COLS = ((131, 194), (259, 322))


# ---------------------------------------------------------------------------
# Workaround: this walrus build only accepts ONE sem wait per instruction.
# After Tile scheduling, split every multi-wait instruction: the overflow
# waits move onto same-engine NoOps inserted immediately before it.
def _split_multi_waits(nc):
    for f in nc.m.functions:
        for bb in f.blocks:
            out = []
            for inst in bb.instructions:
                si = getattr(inst, "sync_info", None)
                if si is not None and si.on_wait and len(si.on_wait) > 1:
                    extras = list(si.on_wait[:-1])
                    si.on_wait = list(si.on_wait[-1:])
                    for w in extras:
                        nop = mybir.InstNoOp(
                            name=f"I-{nc.next_id()}", ins=[], outs=[]
                        )
                        nop.engine = inst.engine
                        nop.sync_info = mybir.SyncInfo(on_wait=[w], on_update=[])
                        out.append(nop)
                out.append(inst)
            bb.instructions[:] = out


# ---------------------------------------------------------------------------
# Device program (identical on every core; per-core data differs)
def _emit_body(nc, tc, dram, ctx_pools, dbg=None):
    xT_d, wqkT_d, wvT_d, woutT_d, y_d = dram
    persist = ctx_pools["persist"]

    # persistent SBUF
    qk_sb = [persist.tile([128, T], F32R, tag=f"qk{i}", name=f"qk{i}") for i in range(FQK // 128)]
    v_sb = persist.tile([128, NT_TILE * VSTRIDE], BF16, tag="v", name="v")
    v_pack = v_sb.rearrange("p (j s) -> p j s", s=VSTRIDE)
    oall_sb = [persist.tile([128, T], BF16, tag=f"oall{i}", name=f"oall{i}") for i in range(DV // 128)]
    wout_sb = [persist.tile([128, E], BF16, tag=f"wout{i}", name=f"wout{i}") for i in range(DV // 128)]
    ones_t = persist.tile([D + 1, 128], F32R, tag="ones_t", name="ones_t")
    # f32r tiles cannot be Memset directly (ISA rejects the value type):
    # constant fills go through an F32 ones scratch + DVE copy/scale, which
    # performs the f32r rounding on write.
    fill_f32 = persist.tile([128, 512], F32, tag="fill_f32", name="fill_f32")
    nc.vector.memset(fill_f32, 1.0)
    # fp8e5 copy of qk_sb for the pass-0 max (score error ~+-23 absolute,
    # well inside the +-85 exp window; the softmax shift cancels exactly).
    qk8_sb = [persist.tile([128, T], F8, tag=f"qk8_{i}", name=f"qk8_{i}") for i in range(FQK // 128)]
    nc.vector.tensor_copy(out=ones_t, in_=fill_f32[0:D + 1, 0:128])

    with (
        tc.tile_pool(name="aug", bufs=2) as augp,
        tc.tile_pool(name="mx", bufs=2) as mxp_pool,
    ):
        # rolling per-head augmented q/k tiles [D+1, T]; built by DMA from
        # qk_sb (DMA shifts partitions; DVE cannot). Two ring buffers: the
        # ones row (k_aug row D) is written once per physical buffer (h<2)
        # and survives row-0:D overwrites for h>=2.
        k_augs, q_augs = {}, {}
        q8s, k8s = {}, {}

        def build_aug(h):
            odd = h % 2 == 1
            q_tile, k_tile = h // 2, 2 + h // 2
            off = D if odd else 0
            q_aug = augp.tile([D + 1, T], F32R, tag="qaug", name=f"qaug{h}")
            k_aug = augp.tile([D + 1, T], F32R, tag="kaug", name=f"kaug{h}")
            nc.sync.dma_start(out=q_aug[0:D, :], in_=qk_sb[q_tile][off:off + D, :])
            nc.sync.dma_start(out=k_aug[0:D, :], in_=qk_sb[k_tile][off:off + D, :])
            if h < 2:
                for s4 in range(4):
                    nc.vector.tensor_copy(
                        out=k_aug[D:D + 1, s4 * 512:(s4 + 1) * 512],
                        in_=fill_f32[0:1, :],
                    )
            q_augs[h], k_augs[h] = q_aug, k_aug
            # fp8 DoubleRow slabs [32, 2, T]: d-halves stacked in dim 1
            q8 = augp.tile([32, 2 * T], F8, tag="q8", name=f"q8_{h}")
            k8 = augp.tile([32, 2 * T], F8, tag="k8", name=f"k8_{h}")
            for half in range(2):
                o8 = off + half * 32
                nc.scalar.dma_start(
                    out=q8[:, half * T:(half + 1) * T], in_=qk8_sb[q_tile][o8:o8 + 32, :]
                )
                nc.scalar.dma_start(
                    out=k8[:, half * T:(half + 1) * T], in_=qk8_sb[k_tile][o8:o8 + 32, :]
                )
            q8s[h] = q8.rearrange("p (two t) -> p two t", two=2)
            k8s[h] = k8.rearrange("p (two t) -> p two t", two=2)

        # ---- pass-0 (exact per-query max) emission helpers --------------
        # One c-block = scores for one 128-query i-tile against all 2048
        # keys ([i, j] orientation, q-stationary) + two wide PSUM reduces.
        # mxp collects 8 partial maxes per ic-chunk; the chunk-end combine
        # writes the NEGATED max and DMAs it (transposing AP) into q_aug
        # row D, where the main-pass augmented matmul applies the shift.
        mx_state = {}

        def pass0_block(pxp, h, ic, c):
            q_aug = q_augs[h]
            if c == 0:
                mx_state[h] = mxp_pool.tile(
                    [128, 2 * NT_CHUNK], F32, tag="mxp", name=f"mxp{h}_{ic}"
                )
            mxp = mx_state[h]
            it = ic * NT_CHUNK + c
            wide = [
                pxp.tile([128, 1024], F32, tag="px", name=f"px{h}_{it}_{half}")
                for half in range(2)
            ]
            q8, k8 = q8s[h], k8s[h]
            for jc in range(NT_CHUNK):
                nc.tensor.matmul(
                    wide[jc // 2][:, (jc % 2) * 512:(jc % 2 + 1) * 512],
                    q8[:, :, it * 128:(it + 1) * 128],
                    k8[:, :, jc * 512:(jc + 1) * 512],
                    start=True,
                    stop=True,
                    perf_mode=DR,
                )
            nc.vector.reduce_max(out=mxp[:, 2 * c:2 * c + 1], in_=wide[0], axis=AX)
            nc.vector.reduce_max(out=mxp[:, 2 * c + 1:2 * c + 2], in_=wide[1], axis=AX)
            if c == NT_CHUNK - 1:
                mneg = mxp_pool.tile([128, NT_CHUNK], F32R, tag="mneg", name=f"mneg{h}_{ic}")
                nc.vector.tensor_reduce(
                    out=mneg,
                    in_=mxp.rearrange("p (c t) -> p c t", t=2),
                    axis=AX,
                    op=mybir.AluOpType.max,
                    negate=True,
                )
                for cc in range(NT_CHUNK):
                    seg = ic * 512 + cc * 128
                    nc.sync.dma_start(
                        out=q_aug[D:D + 1, seg:seg + 128],
                        in_=mneg[:, cc:cc + 1],
                    )

        # ---- main-pass emission helper -----------------------------------
        def main_chunk(psp, pop, prp, ptp, rrp, h, ic, pass0_h, pxp, tail=None):
            """One 512-query chunk of head h's attention; optionally emits
            pass-0 c-blocks of head pass0_h after every 4th j-tile."""
            q_aug, k_aug = q_augs[h], k_augs[h]
            odd = h % 2 == 1
            o_base = D if odd else 0
            den = 0 if odd else D
            stat0, stat1 = V_STAT[h]
            ics = slice(ic * 512, (ic + 1) * 512)
            po = pop.tile([128, 512], F32, tag="po", name=f"po{h}_{ic}")
            nj = stat1 - stat0
            for jt in range(NT_TILE):
                ps = psp.tile([128, 512], F32, tag="ps", name=f"ps{h}_{ic}_{jt}")
                nc.tensor.matmul(
                    ps,
                    k_aug[:, jt * 128:(jt + 1) * 128],
                    q_aug[:, ics],
                    start=True,
                    stop=True,
                )
                pT = ptp.tile([128, 512], BF16, tag="pt", name=f"pt{h}_{ic}_{jt}")
                nc.scalar.activation(
                    out=pT, in_=ps, func=mybir.ActivationFunctionType.Exp
                )
                nc.tensor.matmul(
                    po[0:nj, :] if nj < 128 else po,
                    v_pack[:, jt, stat0:stat1],
                    pT,
                    start=(jt == 0),
                    stop=(jt == NT_TILE - 1),
                )
                if pass0_h is not None and jt % NT_CHUNK == NT_CHUNK - 1:
                    pass0_block(pxp, pass0_h, ic, jt // NT_CHUNK)
            # normalize: r = 1/denominator, replicate across partitions via
            # ones outer-product matmul (PE), multiply straight out of PSUM.
            r = rrp.tile([D + 1, 512], F32R, tag="r", name=f"r{h}_{ic}")
            with nc.allow_low_precision(reason="f32r reciprocal: ~2^-19 rounding on 1/den"):
                nc.vector.reciprocal(out=r[den:den + 1, :], in_=po[den:den + 1, :])
            pr = prp.tile([128, 512], F32, tag="pr", name=f"pr{h}_{ic}")
            nc.tensor.matmul(
                pr,
                ones_t[den:den + 1, 0:128],
                r[den:den + 1, :],
                start=True,
                stop=True,
            )
            ot = rrp.tile([128, 512], F32, tag="ot", name=f"ot{h}_{ic}")
            nc.scalar.copy(
                out=ot[o_base:o_base + D, :], in_=po[o_base:o_base + D, :]
            )
            nc.vector.tensor_mul(
                oall_sb[h // 2][o_base:o_base + D, ics],
                ot[o_base:o_base + D, :],
                pr[o_base:o_base + D, :],
            )
            if tail is not None:
                tail()

        # ==================================================================
        with tc.tile_pool(name="px", bufs=2, space="PSUM") as pxp:
            # ---- Phase 1a: qk^T projection (feature-major) --------------
            with tc.tile_pool(name="xtwv", bufs=1) as xtwv:
              with (
                tc.tile_pool(name="wqkp", bufs=1) as wqkp,
                tc.tile_pool(name="pj", bufs=4, space="PSUM") as pj,
              ):
                xt_sb = [xtwv.tile([128, T], F32R, tag=f"xt{i}", name=f"xt{i}") for i in range(NE)]
                wqk_sb = [wqkp.tile([128, FQK], F32R, tag=f"wqk{i}", name=f"wqk{i}") for i in range(NE)]
                wv_sb = [xtwv.tile([128, DV], F32R, tag=f"wv{i}", name=f"wv{i}") for i in range(NE)]
                # x/wqk feed phase 1a immediately; wv only feeds phase 1b.
                # Spread across the three DMA-capable queues, first-needed first;
                # x0 is split so the first matmul can start after ~1/2 tile.
                qs = [nc.sync, nc.scalar, nc.gpsimd]
                qs[0].dma_start(out=xt_sb[0][:, 0:1024], in_=xT_d[0:128, 0:1024])
                qs[1].dma_start(out=wqk_sb[0], in_=wqkT_d[0:128, :])
                qs[2].dma_start(out=xt_sb[0][:, 1024:2048], in_=xT_d[0:128, 1024:2048])
                for i in range(1, NE):
                    qs[i % 3].dma_start(out=xt_sb[i], in_=xT_d[i * 128:(i + 1) * 128, :])
                    qs[(i + 1) % 3].dma_start(out=wqk_sb[i], in_=wqkT_d[i * 128:(i + 1) * 128, :])
                for i in range(NE):
                    qs[(i + 2) % 3].dma_start(out=wv_sb[i], in_=wvT_d[i * 128:(i + 1) * 128, :])
                for i in range(DV // 128):
                    qs[i % 3].dma_start(out=wout_sb[i], in_=woutT_d[i * 128:(i + 1) * 128, :])

                for ff in range(FQK // 128):
                    ps4 = [pj.tile([128, 512], F32, tag="pj", name=f"pj{ff}_{i}") for i in range(NT_CHUNK)]
                    for ne in range(NE):
                        lhsT = wqk_sb[ne][:, ff * 128:(ff + 1) * 128]
                        for tt in range(NT_CHUNK):
                            nc.tensor.matmul(
                                ps4[tt],
                                lhsT,
                                xt_sb[ne][:, tt * 512:(tt + 1) * 512],
                                start=(ne == 0),
                                stop=(ne == NE - 1),
                            )
                    for tt in range(NT_CHUNK):
                        nc.vector.tensor_copy(
                            out=qk_sb[ff][:, tt * 512:(tt + 1) * 512], in_=ps4[tt]
                        )
                # fp8 stage for pass-0 (gpsimd converts dtype on copy)
                for i in range(FQK // 128):
                    for tth in range(2):
                        nc.gpsimd.tensor_copy(
                            out=qk8_sb[i][:, tth * 1024:(tth + 1) * 1024],
                            in_=qk_sb[i][:, tth * 1024:(tth + 1) * 1024],
                        )
                # aug tiles need q (ff0/ff1) and k (ff2/ff3) slabs complete
                build_aug(0)
                build_aug(1)

                # v constants: ones + zero columns pre-planted once
                for col in ONES_COLS:
                    nc.vector.tensor_copy(
                        out=v_pack[:, :, col:col + 1],
                        in_=fill_f32[:, 0:NT_TILE].rearrange("p (a b) -> p a b", b=1),
                    )
                for lo, hi in ZERO_COLS:
                    w = hi - lo
                    for jh in range(2):
                        nc.vector.tensor_scalar_mul(
                            v_pack[:, jh * 8:(jh + 1) * 8, lo:hi],
                            fill_f32[:, 0:8 * w].rearrange("p (a b) -> p a b", b=w),
                            0.0,
                        )

              # ---- Phase 1b: v projection (token-major) x pass-0(head 0) --
              with tc.tile_pool(name="pv", bufs=2, space="PSUM") as pv:
                for tj in range(NT_TILE):
                    psv = pv.tile([128, DV], F32, tag="pv", name=f"pv{tj}")
                    for ne in range(NE):
                        nc.tensor.matmul(
                            psv,
                            xt_sb[ne][:, tj * 128:(tj + 1) * 128],
                            wv_sb[ne],
                            start=(ne == 0),
                            stop=(ne == NE - 1),
                        )
                    for h in range(HPG):
                        nc.scalar.copy(
                            out=v_pack[:, tj, V_OFF[h]:V_OFF[h] + D],
                            in_=psv[:, h * D:(h + 1) * D],
                        )
                    pass0_block(pxp, 0, tj // NT_CHUNK, tj % NT_CHUNK)

            # ---- Phase 2: heads 0..2 main x pass-0(h+1) ------------------
            with (
                tc.tile_pool(name="ps", bufs=2, space="PSUM") as psp,
                tc.tile_pool(name="po", bufs=1, space="PSUM") as pop,
                tc.tile_pool(name="pr", bufs=1, space="PSUM") as prp,
                tc.tile_pool(name="pt", bufs=4) as ptp,
                tc.tile_pool(name="rr", bufs=2) as rrp,
            ):
                for h in range(HPG - 1):
                    for ic in range(NT_CHUNK):
                        main_chunk(psp, pop, prp, ptp, rrp, h, ic, h + 1, pxp)
                    if h + 2 < HPG:
                        build_aug(h + 2)

        # ---- Phase 3: head 3 main x partial out-projection ---------------
        with (
            tc.tile_pool(name="ps3", bufs=2, space="PSUM") as psp3,
            tc.tile_pool(name="po3", bufs=1, space="PSUM") as pop3,
            tc.tile_pool(name="pr3", bufs=1, space="PSUM") as prp3,
            tc.tile_pool(name="py", bufs=4, space="PSUM") as pyp,
            tc.tile_pool(name="pt3", bufs=4) as ptp3,
            tc.tile_pool(name="rr3", bufs=2) as rrp3,
            tc.tile_pool(name="ysb", bufs=3) as ysbp,
        ):
            h = HPG - 1

            def outproj_tail(ic):
                for tt in range(ic * NT_CHUNK, (ic + 1) * NT_CHUNK):
                    pys = [pyp.tile([128, 512], F32, tag="py", name=f"py{tt}_{oc}") for oc in range(2)]
                    for es in range(DV // 128):
                        lhsT = oall_sb[es][:, tt * 128:(tt + 1) * 128]
                        for oc in range(2):
                            nc.tensor.matmul(
                                pys[oc],
                                lhsT,
                                wout_sb[es][:, oc * 512:(oc + 1) * 512],
                                start=(es == 0),
                                stop=(es == DV // 128 - 1),
                            )
                    yt = ysbp.tile([128, E], F32, tag="y", name=f"y{tt}")
                    nc.vector.tensor_copy(out=yt[:, 0:512], in_=pys[0])
                    nc.scalar.copy(out=yt[:, 512:1024], in_=pys[1])
                    nc.sync.dma_start(out=y_d[tt * 128:(tt + 1) * 128, :], in_=yt)

            for ic in range(NT_CHUNK):
                main_chunk(
                    psp3, pop3, prp3, ptp3, rrp3, h, ic, None, None,
                    tail=(lambda ic=ic: outproj_tail(ic)),
                )


def _build_nc(reps=1, debug=False):
    nc = bass.Bass()
    xT_d = nc.declare_dram_parameter("xT", [E, T], F32R, isOutput=False)
    wqkT_d = nc.declare_dram_parameter("wqkT", [E, FQK], F32R, isOutput=False)
    wvT_d = nc.declare_dram_parameter("wvT", [E, DV], F32R, isOutput=False)
    woutT_d = nc.declare_dram_parameter("woutT", [DV, E], BF16, isOutput=False)
    y_d = nc.declare_dram_parameter("y", [T, E], F32, isOutput=True)
    dram = (xT_d, wqkT_d, wvT_d, woutT_d, y_d)
    with tile_mod.TileContext(nc) as tc:
        for _ in range(reps):
            with tc.tile_pool(name="persist", bufs=1) as persist:
                _emit_body(nc, tc, dram, {"persist": persist}, dbg=None)
    _split_multi_waits(nc)
    return nc


# ---------------------------------------------------------------------------
# Execution: cached jitted shard_map over 8 cores (axon/PJRT path)
_RUNNERS = {}


class _Runner:
    def __init__(self, reps=1, debug=False):
        import jax
        from jax.sharding import Mesh, PartitionSpec
        from jax.experimental.shard_map import shard_map
        from concourse import bass2jax

        bass2jax.install_neuronx_cc_hook()
        nc = self._nc = _build_nc(reps, debug=debug)

        partition_name = (
            nc.partition_id_tensor.name if nc.partition_id_tensor else None
        )
        in_names, out_names, out_avals, zero_outs = [], [], [], []
        for alloc in nc.m.functions[0].allocations:
            if not isinstance(alloc, mybir.MemoryLocationSet):
                continue
            name = alloc.memorylocations[0].name
            if alloc.kind == "ExternalInput":
                if name != partition_name:
                    in_names.append(name)
            elif alloc.kind == "ExternalOutput":
                shape = tuple(alloc.tensor_shape)
                dtype = mybir.dt.np(alloc.dtype)
                out_names.append(name)
                out_avals.append(jax.core.ShapedArray(shape, dtype))
                zero_outs.append(np.zeros(shape, dtype))
        self.in_names, self.out_names = in_names, out_names
        self.out_avals, self.zero_outs = out_avals, zero_outs
        n_params, n_outs = len(in_names), len(out_names)
        all_in_names = list(in_names) + list(out_names)
        if partition_name is not None:
            all_in_names.append(partition_name)
        all_in_names = tuple(all_in_names)

        def _body(*args):
            operands = list(args)
            if partition_name is not None:
                operands.append(bass2jax.partition_id_tensor())
            outs = bass2jax._bass_exec_p.bind(
                *operands,
                out_avals=tuple(out_avals),
                in_names=all_in_names,
                out_names=tuple(out_names),
                lowering_input_output_aliases=(),
                sim_require_finite=True,
                sim_require_nnan=True,
                nc=nc,
            )
            return tuple(outs)

        devices = jax.devices()[:N_CORES]
        assert len(devices) == N_CORES
        self.mesh = Mesh(np.asarray(devices), ("core",))
        in_specs = (PartitionSpec("core"),) * (n_params + n_outs)
        out_specs = (PartitionSpec("core"),) * n_outs
        self.donate = tuple(range(n_params, n_params + n_outs))
        self.sharded = jax.jit(
            shard_map(
                _body,
                mesh=self.mesh,
                in_specs=in_specs,
                out_specs=out_specs,
                check_rep=False,
            ),
            donate_argnums=self.donate,
            keep_unused=True,
        )

    def stage_inputs(self, per_core_in):
        """per_core_in: list of dicts (len N_CORES) -> device-resident concat arrays."""
        import jax
        from jax.sharding import NamedSharding, PartitionSpec

        sh = NamedSharding(self.mesh, PartitionSpec("core"))
        staged = []
        for name in self.in_names:
            cat = np.concatenate(
                [np.asarray(per_core_in[c][name]) for c in range(N_CORES)], axis=0
            )
            staged.append(jax.device_put(cat, sh))
        return staged

    def fresh_outs(self):
        import jax
        from jax.sharding import NamedSharding, PartitionSpec

        sh = NamedSharding(self.mesh, PartitionSpec("core"))
        return [
            jax.device_put(
                np.zeros((N_CORES * z.shape[0], *z.shape[1:]), z.dtype), sh
            )
            for z in self.zero_outs
        ]

    def run(self, staged_in, out_bufs):
        import jax

        outs = self.sharded(*staged_in, *out_bufs)
        jax.block_until_ready(outs)
        return outs

    def results(self, outs):
        res = []
        for c in range(N_CORES):
            d = {}
            for i, name in enumerate(self.out_names):
                full = np.asarray(outs[i])
                d[name] = full.reshape(N_CORES, *self.out_avals[i].shape)[c]
            res.append(d)
        return res


def _get_runner(reps=1):
    if reps not in _RUNNERS:
        _RUNNERS[reps] = _Runner(reps)
    return _RUNNERS[reps]


# ---------------------------------------------------------------------------
# Host-side sharding / gather
def _per_core_inputs(x, w_qkv, w_out):
    x = np.asarray(x, dtype=np.float32)
    w_qkv = np.asarray(w_qkv, dtype=np.float32)
    w_out = np.asarray(w_out, dtype=np.float32)
    per_core = []
    for c in range(N_CORES):
        b, g = c // GROUPS, c % GROUPS
        hs = np.arange(g * HPG, (g + 1) * HPG)
        # qkv reshape order in reference: f = d*48 + k*16 + h
        rows_q = (np.arange(D)[None, :] * (3 * H_TOTAL) + hs[:, None]).reshape(-1)
        rows_k = rows_q + H_TOTAL
        rows_v = rows_q + 2 * H_TOTAL
        wqk = np.concatenate([w_qkv[rows_q], SCALE * w_qkv[rows_k]], axis=0)
        per_core.append(
            {
                "xT": np.ascontiguousarray(x[b].T),
                "wqkT": np.ascontiguousarray(wqk.T),
                "wvT": np.ascontiguousarray(w_qkv[rows_v].T),
                "woutT": np.ascontiguousarray(
                    w_out[:, g * DV:(g + 1) * DV].T
                ).astype(mybir.dt.np(mybir.dt.bfloat16)),
            }
        )
    return per_core


def kernel(x, w_qkv, w_out):
    runner = _get_runner(1)
    staged = runner.stage_inputs(_per_core_inputs(x, w_qkv, w_out))
    outs = runner.run(staged, runner.fresh_outs())
    res = runner.results(outs)
    y = np.zeros((B, T, E), dtype=np.float64)
    for c in range(N_CORES):
        y[c // GROUPS] += res[c]["y"].astype(np.float64)
    return y.astype(np.float32)
